# revision 3
# baseline (speedup 1.0000x reference)
"""CorrectedLinear on 8 TRN2 NeuronCores.

Math: out = x @ W.T + b + (x @ V_r) @ C.T
    = x @ (W.T + V_r @ C.T) + b          -- fold the rank-32 correction
      into a single effective weight matrix Wt [d_in, d_out] (0.05% of
      the GEMM FLOPs, done host-side in float64).

Sharding: pure data-parallel over the batch dim (8 batches -> 8 cores).
Each core computes a [8192, 1024] x [1024, 1024] GEMM.

Device layout: the PE contracts along the partition axis, so both
operands need d_in on partitions. x is fed pre-transposed per core
(xT [d_in, t]) and the output is produced transposed (outT [d_out, t]),
un-transposed on the host. All matmul operands use float32r (full-rate
fp32, ~1.5e-4 matmul precision) with fp32 PSUM accumulation.

Weights are fed as 64 contiguous [128, 128] blocks ordered so the
blocks needed by the first output group arrive first; the first token
chunk is split in half so the first matmul group is gated on ~1.5 MB of
DMA instead of ~6.5 MB.
"""

import numpy as np

N_CORES = 8
T = 8192          # tokens per core (batch entry)
D = 1024          # d_in
O = 1024          # d_out
TCH = 512         # moving free dim per matmul (= one PSUM bank of fp32)
NT = T // TCH     # 16 t-chunks
ND = D // 128     # 8 contraction slices
NO = O // 128     # 8 output-partition slices

_nc = None


def _build():
    import concourse.bacc as bacc
    import concourse.mybir as mybir
    import concourse.tile as tile

    f32 = mybir.dt.float32
    f32r = mybir.dt.float32r

    nc = bacc.Bacc(
        "TRN2", target_bir_lowering=False, debug=False, num_devices=N_CORES
    )
    xT_d = nc.dram_tensor("xT", [D, T], f32r, kind="ExternalInput")
    Wb_d = nc.dram_tensor("Wb", [NO, ND, 128, 128], f32r, kind="ExternalInput")
    b_d = nc.dram_tensor("bb", [O], f32, kind="ExternalInput")
    outT_d = nc.dram_tensor("outT", [O, T], f32, kind="ExternalOutput")

    with tile.TileContext(nc) as tc:
        with (
            tc.tile_pool(name="wt", bufs=1) as wt_pool,
            tc.tile_pool(name="bp", bufs=1) as b_pool,
            tc.tile_pool(name="x0p", bufs=16) as x0_pool,
            tc.tile_pool(name="xp", bufs=24) as x_pool,
            tc.tile_pool(name="op", bufs=6) as o_pool,
            tc.tile_pool(name="ps", bufs=4, space="PSUM") as psum_pool,
        ):
            # --- weight blocks, o-group 0 first, then chunk-0 x, then rest
            wts = [[None] * ND for _ in range(NO)]
            for o in range(NO):
                for d in range(ND):
                    w = wt_pool.tile(
                        [128, 128], f32r, name=f"w{o}_{d}", tag=f"w{o}_{d}"
                    )
                    nc.sync.dma_start(out=w[:], in_=Wb_d.ap()[o, d])
                    wts[o][d] = w
                if o == 0:
                    # chunk 0, in two half-chunks of 256 tokens
                    x0 = [[None] * ND for _ in range(2)]
                    for h in range(2):
                        for d in range(ND):
                            xh = x0_pool.tile(
                                [128, TCH // 2], f32r, name="x0", tag="x0"
                            )
                            nc.sync.dma_start(
                                out=xh[:],
                                in_=xT_d.ap()[
                                    d * 128 : (d + 1) * 128,
                                    h * (TCH // 2) : (h + 1) * (TCH // 2),
                                ],
                            )
                            x0[h][d] = xh

            b_sb = b_pool.tile([128, NO], f32, name="b_sb")
            nc.sync.dma_start(
                out=b_sb[:], in_=b_d.ap().rearrange("(j p) -> p j", p=128)
            )

            def evac_and_store(o, acc, t0, tn):
                """PSUM -> SBUF (+bias) -> DRAM, alternating ACT/DVE."""
                ot = o_pool.tile([128, tn], f32, name="ot", tag="ot")
                if o % 2 == 0:
                    nc.scalar.activation(
                        ot[:],
                        acc[:],
                        mybir.ActivationFunctionType.Identity,
                        bias=b_sb[:, o : o + 1],
                    )
                else:
                    nc.vector.tensor_scalar_add(ot[:], acc[:], b_sb[:, o : o + 1])
                nc.scalar.dma_start(
                    out=outT_d.ap()[o * 128 : (o + 1) * 128, t0 : t0 + tn],
                    in_=ot[:],
                )

            # --- chunk 0: two half-chunks, o-major so group (o=0, h=0)
            # only needs the o=0 weight blocks + 1 MB of x
            for h in range(2):
                for o in range(NO):
                    acc = psum_pool.tile([128, TCH // 2], f32, name="acc", tag="acc")
                    for d in range(ND):
                        nc.tensor.matmul(
                            acc[:],
                            wts[o][d][:],
                            x0[h][d][:],
                            start=(d == 0),
                            stop=(d == ND - 1),
                        )
                    evac_and_store(o, acc, h * (TCH // 2), TCH // 2)

            # --- steady state: full 512-token chunks
            for t in range(1, NT):
                xs = []
                for d in range(ND):
                    xt = x_pool.tile([128, TCH], f32r, name="xt", tag="xt")
                    nc.sync.dma_start(
                        out=xt[:],
                        in_=xT_d.ap()[
                            d * 128 : (d + 1) * 128, t * TCH : (t + 1) * TCH
                        ],
                    )
                    xs.append(xt)
                for o in range(NO):
                    acc = psum_pool.tile([128, TCH], f32, name="acc", tag="acc")
                    for d in range(ND):
                        nc.tensor.matmul(
                            acc[:],
                            wts[o][d][:],
                            xs[d][:],
                            start=(d == 0),
                            stop=(d == ND - 1),
                        )
                    evac_and_store(o, acc, t * TCH, TCH)
    nc.compile()
    return nc


def _get_nc():
    global _nc
    if _nc is None:
        _nc = _build()
    return _nc


def _make_in_maps(x, W, b, V_r, C):
    Wt = (
        W.astype(np.float64).T + V_r.astype(np.float64) @ C.astype(np.float64).T
    ).astype(np.float32)
    # [NO, ND, 128, 128] blocks: Wb[o, d] = Wt[128d:128d+128, 128o:128o+128]
    Wb = np.ascontiguousarray(
        Wt.reshape(ND, 128, NO, 128).transpose(2, 0, 1, 3)
    )
    b = np.ascontiguousarray(b, dtype=np.float32)
    return [
        {
            "xT": np.ascontiguousarray(x[i].T.astype(np.float32, copy=False)),
            "Wb": Wb,
            "bb": b,
        }
        for i in range(N_CORES)
    ]


def _execute(in_maps, trace=False):
    from concourse.bass_utils import run_bass_kernel_spmd

    return run_bass_kernel_spmd(
        _get_nc(), in_maps, list(range(N_CORES)), trace=trace
    )


def kernel(x, W, b, V_r, C):
    res = _execute(_make_in_maps(x, W, b, V_r, C))
    out = np.empty((N_CORES, T, O), dtype=np.float32)
    for i in range(N_CORES):
        out[i] = res.results[i]["outT"].T
    return out


# revision 6
# speedup vs baseline: 1.0813x; 1.0813x over previous
"""CorrectedLinear on 8 TRN2 NeuronCores.

Math: out = x @ W.T + b + (x @ V_r) @ C.T
    = x @ (W.T + V_r @ C.T) + b          -- fold the rank-32 correction
      into a single effective weight matrix Wt [d_in, d_out] (0.05% of
      the GEMM FLOPs, done host-side in float64).

Sharding: pure data-parallel over the batch dim (8 batches -> 8 cores).
Each core computes a [8192, 1024] x [1024, 1024] GEMM.

Device layout: the PE contracts along the partition axis, so both
operands need d_in on partitions. x is fed pre-transposed per core
(xT [d_in, t]) and the output is produced transposed (outT [d_out, t]),
un-transposed on the host. All matmul operands use float32r (full-rate
fp32, ~1.5e-4 matmul precision) with fp32 PSUM accumulation.

Weights are fed as 64 contiguous [128, 128] blocks ordered so the
blocks needed by the first output group arrive first; the first token
chunk is split in half so the first matmul group is gated on ~1.5 MB of
DMA instead of ~6.5 MB.
"""

import numpy as np

N_CORES = 8
T = 8192          # tokens per core (batch entry)
D = 1024          # d_in
O = 1024          # d_out
TCH = 512         # moving free dim per matmul (= one PSUM bank of fp32)
NT = T // TCH     # 16 t-chunks
ND = D // 128     # 8 contraction slices
NO = O // 128     # 8 output-partition slices

_nc = None


def _build():
    import concourse.bacc as bacc
    import concourse.mybir as mybir
    import concourse.tile as tile

    f32 = mybir.dt.float32
    f32r = mybir.dt.float32r

    nc = bacc.Bacc(
        "TRN2", target_bir_lowering=False, debug=False, num_devices=N_CORES
    )
    xT_d = nc.dram_tensor("xT", [D, T], f32r, kind="ExternalInput")
    Wb_d = nc.dram_tensor("Wb", [NO, ND, 128, 128], f32r, kind="ExternalInput")
    b_d = nc.dram_tensor("bb", [O], f32, kind="ExternalInput")
    outT_d = nc.dram_tensor("outT", [O, T], f32, kind="ExternalOutput")

    with tile.TileContext(nc) as tc:
        with (
            tc.tile_pool(name="wt", bufs=1) as wt_pool,
            tc.tile_pool(name="bp", bufs=1) as b_pool,
            tc.tile_pool(name="x0p", bufs=16) as x0_pool,
            tc.tile_pool(name="xp", bufs=24) as x_pool,
            tc.tile_pool(name="op", bufs=6) as o_pool,
            tc.tile_pool(name="ps", bufs=4, space="PSUM") as psum_pool,
        ):
            # --- weights: one 512 KB DMA per o-group ([128, ND*128] tile),
            # o=0 first, chunk-0 x interleaved right after it so the first
            # matmul group is gated on ~1.5 MB of DMA
            w_os = [None] * NO
            for o in range(NO):
                w = wt_pool.tile([128, ND, 128], f32r, name=f"w{o}", tag=f"w{o}")
                nc.sync.dma_start(
                    out=w[:], in_=Wb_d.ap()[o].rearrange("d p c -> p d c")
                )
                w_os[o] = w
                if o == 0:
                    # chunk 0, in two half-chunks of 256 tokens
                    x0 = [[None] * ND for _ in range(2)]
                    for h in range(2):
                        for d in range(ND):
                            xh = x0_pool.tile(
                                [128, TCH // 2], f32r, name="x0", tag="x0"
                            )
                            nc.sync.dma_start(
                                out=xh[:],
                                in_=xT_d.ap()[
                                    d * 128 : (d + 1) * 128,
                                    h * (TCH // 2) : (h + 1) * (TCH // 2),
                                ],
                            )
                            x0[h][d] = xh
            wts = [
                [w_os[o][:, d] for d in range(ND)]
                for o in range(NO)
            ]

            b_sb = b_pool.tile([128, NO], f32, name="b_sb")
            nc.sync.dma_start(
                out=b_sb[:], in_=b_d.ap().rearrange("(j p) -> p j", p=128)
            )

            def evac_and_store(o, acc, t0, tn):
                """PSUM -> SBUF (+bias) -> DRAM, alternating ACT/DVE."""
                ot = o_pool.tile([128, tn], f32, name="ot", tag="ot")
                if o % 2 == 0:
                    nc.scalar.activation(
                        ot[:],
                        acc[:],
                        mybir.ActivationFunctionType.Identity,
                        bias=b_sb[:, o : o + 1],
                    )
                else:
                    nc.vector.tensor_scalar_add(ot[:], acc[:], b_sb[:, o : o + 1])
                nc.scalar.dma_start(
                    out=outT_d.ap()[o * 128 : (o + 1) * 128, t0 : t0 + tn],
                    in_=ot[:],
                )

            # --- chunk 0: two half-chunks, o-major so group (o=0, h=0)
            # only needs the o=0 weight blocks + 1 MB of x
            for h in range(2):
                for o in range(NO):
                    acc = psum_pool.tile([128, TCH // 2], f32, name="acc", tag="acc")
                    for d in range(ND):
                        nc.tensor.matmul(
                            acc[:],
                            wts[o][d],
                            x0[h][d][:],
                            start=(d == 0),
                            stop=(d == ND - 1),
                        )
                    evac_and_store(o, acc, h * (TCH // 2), TCH // 2)

            # --- steady state: full 512-token chunks
            for t in range(1, NT):
                xs = []
                for d in range(ND):
                    xt = x_pool.tile([128, TCH], f32r, name="xt", tag="xt")
                    nc.sync.dma_start(
                        out=xt[:],
                        in_=xT_d.ap()[
                            d * 128 : (d + 1) * 128, t * TCH : (t + 1) * TCH
                        ],
                    )
                    xs.append(xt)
                for o in range(NO):
                    acc = psum_pool.tile([128, TCH], f32, name="acc", tag="acc")
                    for d in range(ND):
                        nc.tensor.matmul(
                            acc[:],
                            wts[o][d],
                            xs[d][:],
                            start=(d == 0),
                            stop=(d == ND - 1),
                        )
                    evac_and_store(o, acc, t * TCH, TCH)
    nc.compile()
    return nc


def _get_nc():
    global _nc
    if _nc is None:
        _nc = _build()
    return _nc


def _make_in_maps(x, W, b, V_r, C):
    Wt = (
        W.astype(np.float64).T + V_r.astype(np.float64) @ C.astype(np.float64).T
    ).astype(np.float32)
    # [NO, ND, 128, 128] blocks: Wb[o, d] = Wt[128d:128d+128, 128o:128o+128]
    Wb = np.ascontiguousarray(
        Wt.reshape(ND, 128, NO, 128).transpose(2, 0, 1, 3)
    )
    b = np.ascontiguousarray(b, dtype=np.float32)
    return [
        {
            "xT": np.ascontiguousarray(x[i].T.astype(np.float32, copy=False)),
            "Wb": Wb,
            "bb": b,
        }
        for i in range(N_CORES)
    ]


def _execute(in_maps, trace=False):
    from concourse.bass_utils import run_bass_kernel_spmd

    return run_bass_kernel_spmd(
        _get_nc(), in_maps, list(range(N_CORES)), trace=trace
    )


def kernel(x, W, b, V_r, C):
    res = _execute(_make_in_maps(x, W, b, V_r, C))
    out = np.empty((N_CORES, T, O), dtype=np.float32)
    for i in range(N_CORES):
        out[i] = res.results[i]["outT"].T
    return out


# revision 7
# speedup vs baseline: 1.1394x; 1.0538x over previous
"""CorrectedLinear on 8 TRN2 NeuronCores.

Math: out = x @ W.T + b + (x @ V_r) @ C.T
    = x @ (W.T + V_r @ C.T) + b          -- fold the rank-32 correction
      into a single effective weight matrix Wt [d_in, d_out] (0.05% of
      the GEMM FLOPs, done host-side in float64).

Sharding: pure data-parallel over the batch dim (8 batches -> 8 cores).
Each core computes a [8192, 1024] x [1024, 1024] GEMM.

Device layout: the PE contracts along the partition axis, so both
operands need d_in on partitions. x is fed pre-transposed per core
(xT [d_in, t]) and the output is produced transposed (outT [d_out, t]),
un-transposed on the host. All matmul operands use float32r (full-rate
fp32, ~1.5e-4 matmul precision) with fp32 PSUM accumulation.

Weights are fed as 64 contiguous [128, 128] blocks ordered so the
blocks needed by the first output group arrive first; the first token
chunk is split in half so the first matmul group is gated on ~1.5 MB of
DMA instead of ~6.5 MB.
"""

import numpy as np

N_CORES = 8
T = 8192          # tokens per core (batch entry)
D = 1024          # d_in
O = 1024          # d_out
TCH = 512         # moving free dim per matmul (= one PSUM bank of fp32)
NT = T // TCH     # 16 t-chunks
ND = D // 128     # 8 contraction slices
NO = O // 128     # 8 output-partition slices

_nc = None


def _build():
    import concourse.bacc as bacc
    import concourse.mybir as mybir
    import concourse.tile as tile

    f32 = mybir.dt.float32
    f32r = mybir.dt.float32r

    nc = bacc.Bacc(
        "TRN2", target_bir_lowering=False, debug=False, num_devices=N_CORES
    )
    xT_d = nc.dram_tensor("xT", [D, T], f32r, kind="ExternalInput")
    Wb_d = nc.dram_tensor("Wb", [NO, 128, ND * 128], f32r, kind="ExternalInput")
    b_d = nc.dram_tensor("bb", [O], f32, kind="ExternalInput")
    outT_d = nc.dram_tensor("outT", [O, T], f32, kind="ExternalOutput")

    with tile.TileContext(nc) as tc:
        with (
            tc.tile_pool(name="wt", bufs=1) as wt_pool,
            tc.tile_pool(name="bp", bufs=1) as b_pool,
            tc.tile_pool(name="x0p", bufs=16) as x0_pool,
            tc.tile_pool(name="xp", bufs=24) as x_pool,
            tc.tile_pool(name="op", bufs=6) as o_pool,
            tc.tile_pool(name="ps", bufs=4, space="PSUM") as psum_pool,
        ):
            # --- weights: one 512 KB DMA per o-group ([128, ND*128] tile),
            # o=0 first, chunk-0 x interleaved right after it so the first
            # matmul group is gated on ~1.5 MB of DMA
            b_sb = b_pool.tile([128, NO], f32, name="b_sb")
            nc.sync.dma_start(
                out=b_sb[:], in_=b_d.ap().rearrange("(j p) -> p j", p=128)
            )
            w_os = [None] * NO
            for o in range(NO):
                w = wt_pool.tile([128, ND * 128], f32r, name=f"w{o}", tag=f"w{o}")
                nc.sync.dma_start(
                    out=w[:], in_=Wb_d.ap()[o]
                )
                w_os[o] = w
                if o == 0:
                    # chunk 0, in two half-chunks of 256 tokens
                    x0 = [[None] * ND for _ in range(2)]
                    for h in range(2):
                        for d in range(ND):
                            xh = x0_pool.tile(
                                [128, TCH // 2], f32r, name="x0", tag="x0"
                            )
                            nc.sync.dma_start(
                                out=xh[:],
                                in_=xT_d.ap()[
                                    d * 128 : (d + 1) * 128,
                                    h * (TCH // 2) : (h + 1) * (TCH // 2),
                                ],
                            )
                            x0[h][d] = xh
            wts = [
                [w_os[o][:, d * 128 : (d + 1) * 128] for d in range(ND)]
                for o in range(NO)
            ]


            def evac_and_store(o, acc, t0, tn):
                """PSUM -> SBUF (+bias) -> DRAM, alternating ACT/DVE."""
                ot = o_pool.tile([128, tn], f32, name="ot", tag="ot")
                if o % 2 == 0:
                    nc.scalar.activation(
                        ot[:],
                        acc[:],
                        mybir.ActivationFunctionType.Identity,
                        bias=b_sb[:, o : o + 1],
                    )
                else:
                    nc.vector.tensor_scalar_add(ot[:], acc[:], b_sb[:, o : o + 1])
                nc.scalar.dma_start(
                    out=outT_d.ap()[o * 128 : (o + 1) * 128, t0 : t0 + tn],
                    in_=ot[:],
                )

            # --- chunk 0: two half-chunks, o-major so group (o=0, h=0)
            # only needs the o=0 weight blocks + 1 MB of x
            for h in range(2):
                for o in range(NO):
                    acc = psum_pool.tile([128, TCH // 2], f32, name="acc", tag="acc")
                    for d in range(ND):
                        nc.tensor.matmul(
                            acc[:],
                            wts[o][d],
                            x0[h][d][:],
                            start=(d == 0),
                            stop=(d == ND - 1),
                        )
                    evac_and_store(o, acc, h * (TCH // 2), TCH // 2)

            # --- steady state: full 512-token chunks
            for t in range(1, NT):
                xs = []
                for d in range(ND):
                    xt = x_pool.tile([128, TCH], f32r, name="xt", tag="xt")
                    nc.sync.dma_start(
                        out=xt[:],
                        in_=xT_d.ap()[
                            d * 128 : (d + 1) * 128, t * TCH : (t + 1) * TCH
                        ],
                    )
                    xs.append(xt)
                for o in range(NO):
                    acc = psum_pool.tile([128, TCH], f32, name="acc", tag="acc")
                    for d in range(ND):
                        nc.tensor.matmul(
                            acc[:],
                            wts[o][d],
                            xs[d][:],
                            start=(d == 0),
                            stop=(d == ND - 1),
                        )
                    evac_and_store(o, acc, t * TCH, TCH)
    nc.compile()
    return nc


def _get_nc():
    global _nc
    if _nc is None:
        _nc = _build()
    return _nc


def _make_in_maps(x, W, b, V_r, C):
    Wt = (
        W.astype(np.float64).T + V_r.astype(np.float64) @ C.astype(np.float64).T
    ).astype(np.float32)
    # per-o SBUF image: Wb[o, p, 128d+c] = Wt[128d+p, 128o+c] so each
    # o-slice is one 512 KB DMA with 4 KB contiguous per partition
    Wb = np.ascontiguousarray(
        Wt.reshape(ND, 128, NO, 128).transpose(2, 1, 0, 3).reshape(NO, 128, ND * 128)
    )
    b = np.ascontiguousarray(b, dtype=np.float32)
    return [
        {
            "xT": np.ascontiguousarray(x[i].T.astype(np.float32, copy=False)),
            "Wb": Wb,
            "bb": b,
        }
        for i in range(N_CORES)
    ]


def _execute(in_maps, trace=False):
    from concourse.bass_utils import run_bass_kernel_spmd

    return run_bass_kernel_spmd(
        _get_nc(), in_maps, list(range(N_CORES)), trace=trace
    )


def kernel(x, W, b, V_r, C):
    res = _execute(_make_in_maps(x, W, b, V_r, C))
    out = np.empty((N_CORES, T, O), dtype=np.float32)
    for i in range(N_CORES):
        out[i] = res.results[i]["outT"].T
    return out
